# revision 48
# baseline (speedup 1.0000x reference)
"""CapsuleConv2d (3x3, stride 1, pad 1) with dynamic routing — Trainium2 Bass kernel.

Problem (hardcoded): x (4, 32, 56, 56) f32, weight (4, 4, 9, 8, 16) f32
  -> out (4, 64, 56, 56) f32.

Sharding: 8 cores = 4 batch x 2 pixel-halves of a zero-padded 58x58 grid.
Each core computes all (P_out, P_in) capsule groups for its half of the
padded pixel grid (14 tiles of 128 flat padded pixels); the host unpads
and stitches. Padding-garbage pixels are computed but discarded.

Per-core pipeline (per 128-pixel tile):
  PE    : 9 matmuls (one per conv tap k): stationary = shifted x window
          [32, 128], moving = host-built block-diag weight [32, 256]
          -> priors PSUM [128 pix, 9 k, 256 (o,p,d)]
  DVE/ACT: 3-iteration dynamic routing entirely in free-dim ops
  DMA   : store routed [128 pix, 64 ch] rows; host transposes to NCHW
"""

import sys

sys.path.insert(0, "/opt/trn_rl_repo")

import numpy as np

import concourse.bacc as bacc
import concourse.mybir as mybir
from concourse.bass_utils import run_bass_kernel_spmd
from concourse.hw_specs import get_activation_tables
from concourse.tile import TileContext

# All ACT funcs used here (Square, Ln, Exp) live in act table 6
# ("natural_log_exp_and_others"), but the table-load pass resolves each func
# to its first-containing table, thrashing between tables 0 and 5 (~1.3us per
# reload, ~60 reloads). Offer the pass only table 6 so it emits one load, and
# pin the emitted id to table 6's real index.
_ACT_TABLE_NAME = "natural_log_exp_and_others"


class _PinnedActBacc(bacc.Bacc):
    def insert_act_table_loads(self):
        tabs = get_activation_tables(self.m.arch)
        names = list(tabs.keys())
        idx = names.index(_ACT_TABLE_NAME)
        only = [(_ACT_TABLE_NAME, tabs[_ACT_TABLE_NAME])]
        bacc._bass_rust.insert_act_table_loads(self, only)
        for bb in self.main_func.blocks:
            for inst in bb.instructions:
                if type(inst).__name__ == "InstLoadActFuncSet":
                    if inst.act_func_set_id != idx:
                        inst.act_func_set_id = idx

F32 = mybir.dt.float32
F16 = mybir.dt.float16
AF = mybir.ActivationFunctionType
ALU = mybir.AluOpType
AX = mybir.AxisListType

# geometry
PIN, LIN, POUT, LOUT, KK = 4, 8, 4, 16, 9
CIN = PIN * LIN          # 32
OPD = POUT * PIN * LOUT  # 256 free cols per tap
HP = 58                  # padded grid side
NPIX = HP * HP           # 3364 padded pixels
TILE = 128
NT = 14                  # tiles per core
CORE_PIX = NT * TILE     # 1792
P0_B = NPIX - CORE_PIX   # 1572: second half start
XW_LEN = CORE_PIX + 2 * 59  # 1910: input window incl. tap halo
NCH = POUT * LOUT        # 64 output channels
XIN_LEN = XW_LEN + KK * OPD  # combined input row: x window + weights


def build_program():
    nc = _PinnedActBacc("TRN2", target_bir_lowering=False)
    xin_d = nc.dram_tensor("xin", [CIN, XIN_LEN], F32, kind="ExternalInput")
    out_d = nc.dram_tensor("out", [CORE_PIX, NCH], F32, kind="ExternalOutput")

    with TileContext(nc) as tc:
        with (
            tc.tile_pool(name="const", bufs=1) as const,
            tc.tile_pool(name="pbig", bufs=2) as pbig,
            tc.tile_pool(name="tbig", bufs=1) as tbig,
            tc.tile_pool(name="small", bufs=3) as small,
            tc.tile_pool(name="outp", bufs=2) as outp,
            tc.tile_pool(name="psum_p", bufs=1, space="PSUM") as psum_p,
            tc.tile_pool(name="psum_s", bufs=1, space="PSUM") as psum_s,
        ):
            xin = const.tile([CIN, XIN_LEN], F32)
            # split the load across DMA queues; weights chunk first
            nc.sync.dma_start(out=xin[:, XW_LEN:], in_=xin_d[:, XW_LEN:])
            nchunk = 3
            cs = (XW_LEN + nchunk - 1) // nchunk
            for ci in range(nchunk):
                lo, hi = ci * cs, min((ci + 1) * cs, XW_LEN)
                nc.sync.dma_start(out=xin[:, lo:hi], in_=xin_d[:, lo:hi])
            xw = xin[:, :XW_LEN]
            wm = xin[:, XW_LEN:]
            eps_t = const.tile([TILE, 1], F32, tag="eps")
            nc.vector.memset(eps_t, 1e-30)
            bias_t = {}
            for val in (1.0, 81.0):
                bt = const.tile([TILE, 1], F32, tag=f"bias{int(val)}")
                nc.vector.memset(bt, val)
                bias_t[val] = bt

            def squash_sq(v, sfx):
                """|s|^2 per group: Square (ACT) + reduce_d (DVE)."""
                v2 = small.tile([TILE, OPD], F32, tag="v2" + sfx)
                nc.scalar.activation(out=v2, in_=v, func=AF.Square)
                yield
                sq = small.tile([TILE, 16], F32, tag="sq" + sfx)
                nc.vector.tensor_reduce(
                    out=sq, in_=v2.rearrange("p (g d) -> p g d", d=LOUT),
                    axis=AX.X, op=ALU.add,
                )
                yield
                return sq

            def squash_tail(v, sq, denom_bias, sfx, o_engine=None):
                """Given v = c*s (c = sqrt(denom_bias)) and sq = |v|^2,
                returns outputs = squash(s) = v * sqrt(u)/(u + denom_bias).
                All ACT funcs (Square/Ln/Exp) share one HW table."""
                # g = sqrt(u)/(u+denom_bias) = exp(0.5*ln(u+eps) - ln(u+denom))
                la = small.tile([TILE, 16], F32, tag="la" + sfx)
                nc.scalar.activation(out=la, in_=sq, func=AF.Ln, bias=eps_t[:, :])
                lb = small.tile([TILE, 16], F32, tag="lb" + sfx)
                nc.scalar.activation(
                    out=lb, in_=sq, func=AF.Ln, bias=bias_t[denom_bias][:, :]
                )
                yield
                cc = small.tile([TILE, 16], F32, tag="cc" + sfx)
                nc.vector.scalar_tensor_tensor(
                    out=cc, in0=la, scalar=0.5, in1=lb,
                    op0=ALU.mult, op1=ALU.subtract,
                )
                g = small.tile([TILE, 16], F32, tag="g" + sfx)
                nc.scalar.activation(out=g, in_=cc, func=AF.Exp)
                yield
                o = small.tile([TILE, OPD], F32, tag="o" + sfx)
                (o_engine or nc.vector).tensor_mul(
                    o.rearrange("p (g d) -> p g d", d=LOUT),
                    v.rearrange("p (g d) -> p g d", d=LOUT),
                    g.unsqueeze(2).to_broadcast([TILE, 16, LOUT]),
                )
                yield
                return o

            def squash(v, denom_bias, sfx, o_engine=None):
                sq = yield from squash_sq(v, sfx)
                o = yield from squash_tail(v, sq, denom_bias, sfx, o_engine)
                return o

            def sum_d_tree(t, lr, k0, k1, sfx):
                """lr[:, k0:k1, :] = sum_d t[:, k0:k1, :, :] via fp16 2x
                pairwise adds over the innermost-contiguous d axis."""
                kn = k1 - k0
                tv = t.rearrange("p k (g d) -> p k g d", d=LOUT)[:, k0:k1]
                w1 = tbig.tile([TILE, kn, 16, 8], F16, tag="w1" + sfx)
                nc.vector.tensor_add(w1, tv[:, :, :, 0:8], tv[:, :, :, 8:16])
                yield
                w2 = tbig.tile([TILE, kn, 16, 4], F16, tag="w2" + sfx)
                nc.vector.tensor_add(w2, w1[:, :, :, 0:4], w1[:, :, :, 4:8])
                yield
                w3 = tbig.tile([TILE, kn, 16, 2], F16, tag="w3" + sfx)
                nc.vector.tensor_add(w3, w2[:, :, :, 0:2], w2[:, :, :, 2:4])
                yield
                nc.vector.tensor_add(
                    lr.rearrange("p (k g) -> p k g", k=KK)[:, k0:k1, :],
                    w3[:, :, :, 0],
                    w3[:, :, :, 1],
                )
                yield

            def sum_d_tree_f32(t, lr, k0, k1, sfx, eng):
                """lr[:, k0:k1, :] = sum_d t[:, k0:k1] via f32 pairwise adds
                on `eng` (used to put a reduction on GPSIMD, which has no
                free-axis tensor_reduce)."""
                kn = k1 - k0
                tv = t.rearrange("p k (g d) -> p k g d", d=LOUT)[:, k0:k1]
                w1 = tbig.tile([TILE, kn, 16, 8], F32, tag="x1" + sfx)
                eng.tensor_add(w1, tv[:, :, :, 0:8], tv[:, :, :, 8:16])
                yield
                w2 = tbig.tile([TILE, kn, 16, 4], F32, tag="x2" + sfx)
                eng.tensor_add(w2, w1[:, :, :, 0:4], w1[:, :, :, 4:8])
                yield
                w3 = tbig.tile([TILE, kn, 16, 2], F32, tag="x3" + sfx)
                eng.tensor_add(w3, w2[:, :, :, 0:2], w2[:, :, :, 2:4])
                yield
                eng.tensor_add(
                    lr.rearrange("p (k g) -> p k g", k=KK)[:, k0:k1, :],
                    w3[:, :, :, 0],
                    w3[:, :, :, 1],
                )
                yield

            def logits_contrib(psb, o, sfx, pool_tree=False):
                """sum_d priors * outputs -> [128, 144] laid out (k, op).
                The multiply runs on GPSIMD (split in two k-halves) so the
                DVE reduce pipelines behind the first half; optionally the
                reduce itself runs on GPSIMD as an f32 add-tree."""
                t = tbig.tile([TILE, KK, OPD], F32, tag="tg" + sfx)
                lr = small.tile([TILE, KK * 16], F32, tag="lr" + sfx)
                KH = 5
                for k0, k1 in ((0, KH), (KH, KK)):
                    nc.gpsimd.tensor_mul(
                        t[:, k0:k1, :],
                        psb[:, k0:k1, :],
                        o.unsqueeze(1).to_broadcast([TILE, k1 - k0, OPD]),
                    )
                    yield
                    if pool_tree:
                        yield from sum_d_tree_f32(t, lr, k0, k1, sfx, nc.gpsimd)
                    else:
                        nc.vector.tensor_reduce(
                            out=lr.rearrange("p (k g) -> p k g", k=KK)[
                                :, k0:k1, :
                            ],
                            in_=t[:, k0:k1, :].rearrange(
                                "p k (g d) -> p k g d", d=LOUT
                            ),
                            axis=AX.X, op=ALU.add,
                        )
                        yield
                return lr

            def softmax_k(lg, sfx):
                """softmax over k of [128, 144] (k, op) layout."""
                e = small.tile([TILE, KK * 16], F32, tag="e" + sfx)
                nc.scalar.activation(out=e, in_=lg, func=AF.Exp)
                yield
                z = small.tile([TILE, 16], F32, tag="z" + sfx)
                nc.vector.tensor_reduce(
                    out=z, in_=e.rearrange("p (k g) -> p g k", k=KK),
                    axis=AX.X, op=ALU.add,
                )
                zr = small.tile([TILE, 16], F32, tag="zr" + sfx)
                nc.vector.reciprocal(out=zr, in_=z)
                yield
                pr = small.tile([TILE, KK * 16], F32, tag="pr" + sfx)
                nc.vector.tensor_mul(
                    pr.rearrange("p (k g) -> p k g", k=KK),
                    e.rearrange("p (k g) -> p k g", k=KK),
                    zr.unsqueeze(1).to_broadcast([TILE, KK, 16]),
                )
                yield
                return pr

            def weighted_s(psb, pr, sfx):
                """sum_k probs * priors -> [128, 256] via fp16 tree over k."""
                t = tbig.tile([TILE, KK, OPD], F16, tag="tt" + sfx)
                nc.vector.tensor_mul(
                    t.rearrange("p k (g d) -> p k g d", d=LOUT),
                    psb.rearrange("p k (g d) -> p k g d", d=LOUT),
                    pr.rearrange("p (k g) -> p k g", k=KK)
                    .unsqueeze(3)
                    .to_broadcast([TILE, KK, 16, LOUT]),
                )
                yield
                u1 = tbig.tile([TILE, 4, OPD], F16, tag="u1" + sfx)
                nc.vector.tensor_add(u1, t[:, 0:4, :], t[:, 4:8, :])
                yield
                u2 = tbig.tile([TILE, 2, OPD], F16, tag="u2" + sfx)
                nc.vector.tensor_add(u2, u1[:, 0:2, :], u1[:, 2:4, :])
                yield
                u3 = tbig.tile([TILE, OPD], F16, tag="u3" + sfx)
                nc.vector.tensor_add(u3, u2[:, 0, :], u2[:, 1, :])
                yield
                v = small.tile([TILE, OPD], F32, tag="v" + sfx)
                nc.vector.tensor_add(v, u3, t[:, 8, :])
                yield
                return v

            def tile_body(t, sfx):
                # ---- priors (and their k-sum) via PE ----
                pp = psum_p.tile([TILE, KK, OPD], F32, tag="pp")
                s0 = psum_s.tile([TILE, OPD], F32, tag="s0" + sfx)
                # s0 (k-sum) first: iter-0 squash depends only on it, so
                # routing starts while the per-k priors are still streaming
                for k in range(KK):
                    dj, dk = divmod(k, 3)
                    off = 59 + t * TILE + (dj - 1) * HP + (dk - 1)
                    nc.tensor.matmul(
                        s0,
                        xw[:, off:off + TILE],
                        wm[:, k * OPD:(k + 1) * OPD],
                        start=(k == 0), stop=(k == KK - 1),
                    )
                    yield
                # iter-0 |s|^2 starts as soon as s0 lands, before the pp
                # matmuls are queued on ACT/DVE
                sq0 = yield from squash_sq(s0, sfx)
                for k in range(KK):
                    dj, dk = divmod(k, 3)
                    off = 59 + t * TILE + (dj - 1) * HP + (dk - 1)
                    nc.tensor.matmul(
                        pp[:, k, :],
                        xw[:, off:off + TILE],
                        wm[:, k * OPD:(k + 1) * OPD],
                        start=True, stop=True,
                    )
                    yield
                # priors to SBUF fp16 (ACT) so PSUM frees early, GPSIMD can
                # read, and downstream multiplies run in 16-bit
                psb = pbig.tile([TILE, KK, OPD], F32, tag="psb" + sfx)
                nc.scalar.copy(out=psb, in_=pp)
                yield
                # ---- routing iter 0: probs uniform, s = s0/9; squash
                # folds the 1/9 via denom_bias=81 ----
                o0 = yield from squash_tail(s0, sq0, 81.0, sfx)
                l1 = yield from logits_contrib(psb, o0, sfx)
                # ---- iter 1 ----
                pr1 = yield from softmax_k(l1, sfx)
                v1 = yield from weighted_s(psb, pr1, sfx)
                o1 = yield from squash(v1, 1.0, sfx, o_engine=nc.gpsimd)
                l2c = yield from logits_contrib(psb, o1, sfx)
                l2 = small.tile([TILE, KK * 16], F32, tag="l2" + sfx)
                nc.vector.tensor_add(l2, l1, l2c)
                yield
                # ---- iter 2 ----
                pr2 = yield from softmax_k(l2, sfx)
                v2 = yield from weighted_s(psb, pr2, sfx)
                o2 = yield from squash(v2, 1.0, sfx)
                # ---- sum over input planes p, store [pix, ch] rows ----
                r = outp.tile([TILE, NCH], F32, tag="rr" + sfx)
                nc.vector.tensor_reduce(
                    out=r,
                    in_=o2.rearrange("p (o q d) -> p o d q", o=POUT, q=PIN),
                    axis=AX.X, op=ALU.add,
                )
                yield
                nc.sync.dma_start(
                    out=out_d[t * TILE:(t + 1) * TILE, :], in_=r
                )

            # Interleave instruction emission with a sliding window of two
            # tiles so each engine's in-order queue alternates between two
            # independent dependency chains (fills head-of-line stalls); when
            # a tile finishes emitting, the next one joins mid-flight of its
            # partner (staggered pipeline).
            gens = []
            nxt = 0
            while gens or nxt < NT:
                while len(gens) < 2 and nxt < NT:
                    gens.append(tile_body(nxt, "AB"[nxt % 2]))
                    nxt += 1
                for gn in list(gens):
                    try:
                        next(gn)
                    except StopIteration:
                        gens.remove(gn)
    nc.compile()
    return nc


_PROG = None


def _get_prog():
    global _PROG
    if _PROG is None:
        _PROG = build_program()
    return _PROG


def _make_inputs(x, weight):
    # block-diagonal moving weights: [c=(p,l), (k, o, p, d)]
    wmov = np.zeros((CIN, KK, POUT, PIN, LOUT), np.float32)
    for p in range(PIN):
        # rows p*LIN..p*LIN+LIN-1 hold weight[o, p, k, l, d]
        wmov[p * LIN:(p + 1) * LIN, :, :, p, :] = np.transpose(
            weight[:, p], (2, 1, 0, 3)
        )  # (l, k, o, d) from (o, k, l, d)
    wmov = wmov.reshape(CIN, KK * OPD)

    xp = np.pad(x, ((0, 0), (0, 0), (1, 1), (1, 1))).reshape(4, CIN, NPIX)
    xpm = np.pad(xp, ((0, 0), (0, 0), (64, 64)))
    in_maps = []
    for c in range(8):
        n, half = divmod(c, 2)
        p0 = 0 if half == 0 else P0_B
        lo = 64 + p0 - 59
        xin = np.concatenate([xpm[n][:, lo:lo + XW_LEN], wmov], axis=1)
        in_maps.append({"xin": np.ascontiguousarray(xin)})
    return in_maps


def _assemble(results):
    out = np.empty((4, NCH, 56, 56), np.float32)
    for n in range(4):
        full = np.empty((NCH, NPIX), np.float32)
        full[:, :CORE_PIX] = results[2 * n]["out"].T
        full[:, CORE_PIX:] = results[2 * n + 1]["out"].T[:, CORE_PIX - P0_B:]
        out[n] = full.reshape(NCH, HP, HP)[:, 1:57, 1:57]
    return out


def kernel(x, weight):
    x = np.asarray(x, np.float32)
    weight = np.asarray(weight, np.float32)
    in_maps = _make_inputs(x, weight)
    last_err = None
    for _ in range(3):  # retry transient NRT/device errors
        try:
            res = run_bass_kernel_spmd(
                _get_prog(), in_maps, core_ids=list(range(8))
            )
            return _assemble(res.results)
        except Exception as e:  # noqa: BLE001
            last_err = e
    raise last_err


if __name__ == "__main__":
    rng = np.random.default_rng(0)
    x = rng.standard_normal((4, 32, 56, 56), dtype=np.float32)
    w = rng.standard_normal((4, 4, 9, 8, 16), dtype=np.float32)
    y = kernel(x, w)
    print("out", y.shape, y.dtype, float(np.abs(y).mean()))


# revision 55
# speedup vs baseline: 1.0310x; 1.0310x over previous
"""CapsuleConv2d (3x3, stride 1, pad 1) with dynamic routing — Trainium2 Bass kernel.

Problem (hardcoded): x (4, 32, 56, 56) f32, weight (4, 4, 9, 8, 16) f32
  -> out (4, 64, 56, 56) f32.

Sharding: 8 cores = 4 batch x 2 pixel-halves of a zero-padded 58x58 grid.
Each core computes all (P_out, P_in) capsule groups for its half of the
padded pixel grid (7 super-tiles of 2x128 flat padded pixels); the host
unpads and stitches. Padding-garbage pixels are computed but discarded.

Per-core pipeline (per super-tile = 2 blocks of 128 pixels):
  PE    : per block, 9 matmuls per conv tap (stationary = shifted x window
          [32, 128], moving = host-built block-diag weight [32, 256]) into a
          shared priors PSUM slot + 9 accumulating matmuls for the tap-sum
  ACT   : copies each block's priors PSUM->SBUF (frees PSUM, enables GPSIMD)
  DVE/ACT/GPSIMD: 3-iteration dynamic routing in free-dim ops over both
          blocks at once (2x free-dim per instruction amortizes op overhead);
          fp16 pairwise-add trees for the weighted sum, f32 logits path
  DMA   : store routed [128 pix, 2, 64 ch] rows; host transposes to NCHW
"""

import sys

sys.path.insert(0, "/opt/trn_rl_repo")

import numpy as np

import concourse.bacc as bacc
import concourse.mybir as mybir
from concourse.bass_utils import run_bass_kernel_spmd
from concourse.hw_specs import get_activation_tables
from concourse.tile import TileContext

# All ACT funcs used here (Square, Ln, Exp) live in act table 6
# ("natural_log_exp_and_others"), but the table-load pass resolves each func
# to its first-containing table, thrashing between tables 0 and 5 (~1.3us per
# reload, ~60 reloads). Offer the pass only table 6 so it emits one load, and
# pin the emitted id to table 6's real index.
_ACT_TABLE_NAME = "natural_log_exp_and_others"


class _PinnedActBacc(bacc.Bacc):
    def insert_act_table_loads(self):
        tabs = get_activation_tables(self.m.arch)
        names = list(tabs.keys())
        idx = names.index(_ACT_TABLE_NAME)
        only = [(_ACT_TABLE_NAME, tabs[_ACT_TABLE_NAME])]
        bacc._bass_rust.insert_act_table_loads(self, only)
        for bb in self.main_func.blocks:
            for inst in bb.instructions:
                if type(inst).__name__ == "InstLoadActFuncSet":
                    if inst.act_func_set_id != idx:
                        inst.act_func_set_id = idx


F32 = mybir.dt.float32
F16 = mybir.dt.float16
AF = mybir.ActivationFunctionType
ALU = mybir.AluOpType
AX = mybir.AxisListType

# geometry
PIN, LIN, POUT, LOUT, KK = 4, 8, 4, 16, 9
CIN = PIN * LIN          # 32
OPD = POUT * PIN * LOUT  # 256 free cols per tap
HP = 58                  # padded grid side
NPIX = HP * HP           # 3364 padded pixels
TILE = 128
NB = 2                   # pixel blocks fused per super-tile
NST = 7                  # super-tiles per core
CORE_PIX = NST * NB * TILE   # 1792
P0_B = NPIX - CORE_PIX   # 1572: second half start
XW_LEN = CORE_PIX + 2 * 59  # 1910: input window incl. tap halo
NCH = POUT * LOUT        # 64 output channels
XIN_LEN = XW_LEN + KK * OPD  # combined input row: x window + weights


def build_program():
    nc = _PinnedActBacc("TRN2", target_bir_lowering=False)
    xin_d = nc.dram_tensor("xin", [CIN, XIN_LEN], F32, kind="ExternalInput")
    out_d = nc.dram_tensor("out", [CORE_PIX, NCH], F32, kind="ExternalOutput")

    with TileContext(nc) as tc:
        with (
            tc.tile_pool(name="const", bufs=1) as const,
            tc.tile_pool(name="pbig", bufs=2) as pbig,
            tc.tile_pool(name="tbig", bufs=1) as tbig,
            tc.tile_pool(name="small", bufs=2) as small,
            tc.tile_pool(name="outp", bufs=2) as outp,
            tc.tile_pool(name="psum_p", bufs=1, space="PSUM") as psum_p,
            tc.tile_pool(name="psum_s", bufs=1, space="PSUM") as psum_s,
        ):
            xin = const.tile([CIN, XIN_LEN], F32)
            # split the load across DMA queues; weights chunk first
            nc.sync.dma_start(out=xin[:, XW_LEN:], in_=xin_d[:, XW_LEN:])
            nchunk = 3
            cs = (XW_LEN + nchunk - 1) // nchunk
            for ci in range(nchunk):
                lo, hi = ci * cs, min((ci + 1) * cs, XW_LEN)
                nc.sync.dma_start(out=xin[:, lo:hi], in_=xin_d[:, lo:hi])
            xw = xin[:, :XW_LEN]
            wm = xin[:, XW_LEN:]
            eps_t = const.tile([TILE, 1], F32, tag="eps")
            nc.vector.memset(eps_t, 1e-30)
            bias_t = {}
            for val in (1.0, 81.0):
                bt = const.tile([TILE, 1], F32, tag=f"bias{int(val)}")
                nc.vector.memset(bt, val)
                bias_t[val] = bt

            NG = NB * 16  # squash groups per partition across blocks

            def squash_sq(v, sfx, split=False):
                """|s|^2 per (block, group): Square (ACT) + reduce_d (DVE).
                v: [TILE, NB*OPD]-shaped AP (any space). split=True runs it
                per block so the first block's result lands earlier."""
                v2 = small.tile([TILE, NB * OPD], F32, tag="v2" + sfx)
                sq = small.tile([TILE, NG], F32, tag="sq" + sfx)
                sqv = sq.rearrange("p (b g) -> p b g", b=NB)
                v2v = v2.rearrange("p (b g d) -> p b g d", b=NB, d=LOUT)
                vv = v.rearrange("p (b gd) -> p b gd", b=NB)
                for b in range(NB) if split else (slice(None),):
                    nc.scalar.activation(
                        out=v2v[:, b] if split else v2, in_=vv[:, b] if split else v,
                        func=AF.Square,
                    )
                    yield
                    nc.vector.tensor_reduce(
                        out=sqv[:, b] if split else sq,
                        in_=v2v[:, b] if split
                        else v2.rearrange("p (g d) -> p g d", d=LOUT),
                        axis=AX.X, op=ALU.add,
                    )
                    yield
                return sq

            def squash_tail(v, sq, denom_bias, sfx, o_engine=None):
                """Given v = c*s (c = sqrt(denom_bias)) and sq = |v|^2,
                returns outputs = squash(s) = v * sqrt(u)/(u + denom_bias).
                All ACT funcs (Square/Ln/Exp) share one HW table."""
                # g = sqrt(u)/(u+denom_bias) = exp(0.5*ln(u+eps) - ln(u+denom))
                la = small.tile([TILE, NG], F32, tag="la" + sfx)
                nc.scalar.activation(out=la, in_=sq, func=AF.Ln, bias=eps_t[:, :])
                lb = small.tile([TILE, NG], F32, tag="lb" + sfx)
                nc.scalar.activation(
                    out=lb, in_=sq, func=AF.Ln, bias=bias_t[denom_bias][:, :]
                )
                yield
                cc = small.tile([TILE, NG], F32, tag="cc" + sfx)
                nc.vector.scalar_tensor_tensor(
                    out=cc, in0=la, scalar=0.5, in1=lb,
                    op0=ALU.mult, op1=ALU.subtract,
                )
                g = small.tile([TILE, NG], F32, tag="g" + sfx)
                nc.scalar.activation(out=g, in_=cc, func=AF.Exp)
                yield
                o = small.tile([TILE, NB * OPD], F32, tag="o" + sfx)
                (o_engine or nc.vector).tensor_mul(
                    o.rearrange("p (g d) -> p g d", d=LOUT),
                    v.rearrange("p (g d) -> p g d", d=LOUT),
                    g.unsqueeze(2).to_broadcast([TILE, NG, LOUT]),
                )
                yield
                return o

            def squash(v, denom_bias, sfx, o_engine=None):
                sq = yield from squash_sq(v, sfx)
                o = yield from squash_tail(v, sq, denom_bias, sfx, o_engine)
                return o

            def logits_contrib(psb, o, sfx):
                """sum_d priors * outputs -> [TILE, NB*144] laid out (b,k,op).
                Multiplies on GPSIMD (one per block) pipelined against DVE
                reduces; f32 throughout (logits are precision-sensitive)."""
                t = tbig.tile([TILE, NB, KK, OPD], F32, tag="tg" + sfx)
                lr = small.tile([TILE, NB * KK * 16], F32, tag="lr" + sfx)
                lrv = lr.rearrange("p (b k g) -> p b k g", b=NB, k=KK)
                ov = o.rearrange("p (b gd) -> p b gd", b=NB)
                KH = 5
                for b in range(NB):
                    for k0, k1 in ((0, KH), (KH, KK)):
                        nc.gpsimd.tensor_mul(
                            t[:, b, k0:k1],
                            psb[:, b, k0:k1],
                            ov[:, b].unsqueeze(1)
                            .to_broadcast([TILE, k1 - k0, OPD]),
                        )
                        yield
                        nc.vector.tensor_reduce(
                            out=lrv[:, b, k0:k1],
                            in_=t[:, b, k0:k1].rearrange(
                                "p k (g d) -> p k g d", d=LOUT
                            ),
                            axis=AX.X, op=ALU.add,
                        )
                        yield
                return lr

            def softmax_k(lg, sfx):
                """softmax over k of [TILE, NB*144] in (b, k, op) layout."""
                e = small.tile([TILE, NB * KK * 16], F32, tag="e" + sfx)
                nc.scalar.activation(out=e, in_=lg, func=AF.Exp)
                yield
                z = small.tile([TILE, NG], F32, tag="z" + sfx)
                nc.vector.tensor_reduce(
                    out=z,
                    in_=e.rearrange("p (b k g) -> p b g k", b=NB, k=KK),
                    axis=AX.X, op=ALU.add,
                )
                zr = small.tile([TILE, NG], F32, tag="zr" + sfx)
                nc.vector.reciprocal(out=zr, in_=z)
                yield
                pr = small.tile([TILE, NB * KK * 16], F32, tag="pr" + sfx)
                nc.vector.tensor_mul(
                    pr.rearrange("p (b k g) -> p b k g", b=NB, k=KK),
                    e.rearrange("p (b k g) -> p b k g", b=NB, k=KK),
                    zr.rearrange("p (b g) -> p b g", b=NB)
                    .unsqueeze(2)
                    .to_broadcast([TILE, NB, KK, 16]),
                )
                yield
                return pr

            def weighted_s(psb, pr, sfx):
                """sum_k probs * priors -> [TILE, NB*256] via fp16 tree."""
                t = tbig.tile([TILE, NB, KK, OPD], F16, tag="tt" + sfx)
                nc.vector.tensor_mul(
                    t.rearrange("p b k (g d) -> p (b k) g d", d=LOUT),
                    psb.rearrange("p b k (g d) -> p (b k) g d", d=LOUT),
                    pr.rearrange("p (bk g) -> p bk g", g=16)
                    .unsqueeze(3)
                    .to_broadcast([TILE, NB * KK, 16, LOUT]),
                )
                yield
                u1 = tbig.tile([TILE, NB, 4, OPD], F16, tag="u1" + sfx)
                nc.vector.tensor_add(u1, t[:, :, 0:4, :], t[:, :, 4:8, :])
                yield
                u2 = tbig.tile([TILE, NB, 2, OPD], F16, tag="u2" + sfx)
                nc.vector.tensor_add(u2, u1[:, :, 0:2, :], u1[:, :, 2:4, :])
                yield
                u3 = tbig.tile([TILE, NB, OPD], F16, tag="u3" + sfx)
                nc.vector.tensor_add(u3, u2[:, :, 0, :], u2[:, :, 1, :])
                yield
                v = small.tile([TILE, NB * OPD], F32, tag="v" + sfx)
                nc.vector.tensor_add(
                    v.rearrange("p (b gd) -> p b gd", b=NB), u3, t[:, :, 8, :]
                )
                yield
                return v

            def tile_body(st, sfx):
                # ---- tap-sums s0 for both blocks (iter-0 needs only these) --
                s0 = psum_s.tile([TILE, NB, OPD], F32, tag="s0" + sfx)
                for b in range(NB):
                    t = st * NB + b
                    for k in range(KK):
                        dj, dk = divmod(k, 3)
                        off = 59 + t * TILE + (dj - 1) * HP + (dk - 1)
                        nc.tensor.matmul(
                            s0[:, b],
                            xw[:, off:off + TILE],
                            wm[:, k * OPD:(k + 1) * OPD],
                            start=(k == 0), stop=(k == KK - 1),
                        )
                        yield
                sq0 = yield from squash_sq(
                    s0.rearrange("p b gd -> p (b gd)"), sfx, split=True
                )
                # ---- per-tap priors, block by block through the shared PSUM
                # slot; ACT copies each block out to SBUF fp32 ----
                psb = pbig.tile([TILE, NB, KK, OPD], F32, tag="psb" + sfx)
                for b in range(NB):
                    t = st * NB + b
                    pp = psum_p.tile([TILE, KK, OPD], F32, tag="pp")
                    for k in range(KK):
                        dj, dk = divmod(k, 3)
                        off = 59 + t * TILE + (dj - 1) * HP + (dk - 1)
                        nc.tensor.matmul(
                            pp[:, k, :],
                            xw[:, off:off + TILE],
                            wm[:, k * OPD:(k + 1) * OPD],
                            start=True, stop=True,
                        )
                        yield
                    nc.scalar.copy(out=psb[:, b], in_=pp)
                    yield

                # ---- routing iter 0: probs uniform, s = s0/9; squash folds
                # the 1/9 via denom_bias=81 ----
                o0 = yield from squash_tail(
                    s0.rearrange("p b gd -> p (b gd)"), sq0, 81.0, sfx
                )
                l1 = yield from logits_contrib(psb, o0, sfx)
                # ---- iter 1 ----
                pr1 = yield from softmax_k(l1, sfx)
                v1 = yield from weighted_s(psb, pr1, sfx)
                o1 = yield from squash(v1, 1.0, sfx, o_engine=nc.gpsimd)
                l2c = yield from logits_contrib(psb, o1, sfx)
                l2 = small.tile([TILE, NB * KK * 16], F32, tag="l2" + sfx)
                nc.vector.tensor_add(l2, l1, l2c)
                yield
                # ---- iter 2 ----
                pr2 = yield from softmax_k(l2, sfx)
                v2 = yield from weighted_s(psb, pr2, sfx)
                o2 = yield from squash(v2, 1.0, sfx)
                # ---- sum over input planes p, store [pix, ch] rows ----
                r = outp.tile([TILE, NB, NCH], F32, tag="rr" + sfx)
                nc.vector.tensor_reduce(
                    out=r,
                    in_=o2.rearrange(
                        "p (b o q d) -> p b o d q", b=NB, o=POUT, q=PIN
                    ),
                    axis=AX.X, op=ALU.add,
                )
                yield
                nc.sync.dma_start(
                    out=out_d[st * NB * TILE:(st + 1) * NB * TILE, :]
                    .rearrange("(b p) c -> p b c", b=NB),
                    in_=r,
                )

            # Interleave instruction emission with a sliding window of two
            # super-tiles so each engine's in-order queue alternates between
            # independent dependency chains.
            gens = []
            nxt = 0
            while gens or nxt < NST:
                while len(gens) < 2 and nxt < NST:
                    gens.append(tile_body(nxt, "AB"[nxt % 2]))
                    nxt += 1
                for gn in list(gens):
                    try:
                        next(gn)
                    except StopIteration:
                        gens.remove(gn)
    nc.compile()
    return nc


_PROG = None


def _get_prog():
    global _PROG
    if _PROG is None:
        _PROG = build_program()
    return _PROG


def _make_inputs(x, weight):
    # block-diagonal moving weights: [c=(p,l), (k, o, p, d)]
    wmov = np.zeros((CIN, KK, POUT, PIN, LOUT), np.float32)
    for p in range(PIN):
        # rows p*LIN..p*LIN+LIN-1 hold weight[o, p, k, l, d]
        wmov[p * LIN:(p + 1) * LIN, :, :, p, :] = np.transpose(
            weight[:, p], (2, 1, 0, 3)
        )  # (l, k, o, d) from (o, k, l, d)
    wmov = wmov.reshape(CIN, KK * OPD)

    xp = np.pad(x, ((0, 0), (0, 0), (1, 1), (1, 1))).reshape(4, CIN, NPIX)
    xpm = np.pad(xp, ((0, 0), (0, 0), (64, 64)))
    in_maps = []
    for c in range(8):
        n, half = divmod(c, 2)
        p0 = 0 if half == 0 else P0_B
        lo = 64 + p0 - 59
        xin = np.concatenate([xpm[n][:, lo:lo + XW_LEN], wmov], axis=1)
        in_maps.append({"xin": np.ascontiguousarray(xin)})
    return in_maps


def _assemble(results):
    out = np.empty((4, NCH, 56, 56), np.float32)
    for n in range(4):
        full = np.empty((NCH, NPIX), np.float32)
        full[:, :CORE_PIX] = results[2 * n]["out"].T
        full[:, CORE_PIX:] = results[2 * n + 1]["out"].T[:, CORE_PIX - P0_B:]
        out[n] = full.reshape(NCH, HP, HP)[:, 1:57, 1:57]
    return out


def kernel(x, weight):
    x = np.asarray(x, np.float32)
    weight = np.asarray(weight, np.float32)
    in_maps = _make_inputs(x, weight)
    last_err = None
    for _ in range(3):  # retry transient NRT/device errors
        try:
            res = run_bass_kernel_spmd(
                _get_prog(), in_maps, core_ids=list(range(8))
            )
            return _assemble(res.results)
        except Exception as e:  # noqa: BLE001
            last_err = e
    raise last_err


if __name__ == "__main__":
    rng = np.random.default_rng(0)
    x = rng.standard_normal((4, 32, 56, 56), dtype=np.float32)
    w = rng.standard_normal((4, 4, 9, 8, 16), dtype=np.float32)
    y = kernel(x, w)
    print("out", y.shape, y.dtype, float(np.abs(y).mean()))


# revision 64
# speedup vs baseline: 1.0885x; 1.0558x over previous
"""CapsuleConv2d (3x3, stride 1, pad 1) with dynamic routing — Trainium2 Bass kernel.

Problem (hardcoded): x (4, 32, 56, 56) f32, weight (4, 4, 9, 8, 16) f32
  -> out (4, 64, 56, 56) f32.

Sharding: 8 cores = 4 batch x 2 pixel-halves of a zero-padded 58x58 grid.
Each core computes all (P_out, P_in) capsule groups for its half of the
padded pixel grid (7 super-tiles of 2x128 flat padded pixels); the host
unpads and stitches. Padding-garbage pixels are computed but discarded.

Per-core pipeline (per super-tile = 2 blocks of 128 pixels):
  PE    : per block, 9 matmuls per conv tap (stationary = shifted x window
          [32, 128], moving = host-built block-diag weight [32, 256]) into a
          shared priors PSUM slot + 9 accumulating matmuls for the tap-sum
  ACT   : copies each block's priors PSUM->SBUF (frees PSUM, enables GPSIMD)
  DVE/ACT/GPSIMD: 3-iteration dynamic routing in free-dim ops over both
          blocks at once (2x free-dim per instruction amortizes op overhead);
          fp16 pairwise-add trees for the weighted sum, f32 logits path
  DMA   : store routed [128 pix, 2, 64 ch] rows; host transposes to NCHW
"""

import sys

sys.path.insert(0, "/opt/trn_rl_repo")

import numpy as np

import concourse.bacc as bacc
import concourse.mybir as mybir
from concourse.bass_utils import run_bass_kernel_spmd
from concourse.hw_specs import get_activation_tables
from concourse.tile import TileContext

# All ACT funcs used here (Square, Ln, Exp) live in act table 6
# ("natural_log_exp_and_others"), but the table-load pass resolves each func
# to its first-containing table, thrashing between tables 0 and 5 (~1.3us per
# reload, ~60 reloads). Offer the pass only table 6 so it emits one load, and
# pin the emitted id to table 6's real index.
_ACT_TABLE_NAME = "natural_log_exp_and_others"


class _PinnedActBacc(bacc.Bacc):
    def insert_act_table_loads(self):
        tabs = get_activation_tables(self.m.arch)
        names = list(tabs.keys())
        idx = names.index(_ACT_TABLE_NAME)
        only = [(_ACT_TABLE_NAME, tabs[_ACT_TABLE_NAME])]
        bacc._bass_rust.insert_act_table_loads(self, only)
        for bb in self.main_func.blocks:
            for inst in bb.instructions:
                if type(inst).__name__ == "InstLoadActFuncSet":
                    if inst.act_func_set_id != idx:
                        inst.act_func_set_id = idx


F32 = mybir.dt.float32
F16 = mybir.dt.float16
AF = mybir.ActivationFunctionType
ALU = mybir.AluOpType
AX = mybir.AxisListType

# geometry
PIN, LIN, POUT, LOUT, KK = 4, 8, 4, 16, 9
CIN = PIN * LIN          # 32
OPD = POUT * PIN * LOUT  # 256 free cols per tap
HP = 58                  # padded grid side
NPIX = HP * HP           # 3364 padded pixels
TILE = 128
NB = 2                   # pixel blocks fused per super-tile
NST = 7                  # super-tiles per core
CORE_PIX = NST * NB * TILE   # 1792
P0_B = NPIX - CORE_PIX   # 1572: second half start
XW_LEN = CORE_PIX + 2 * 59  # 1910: input window incl. tap halo
NCH = POUT * LOUT        # 64 output channels
XIN_LEN = XW_LEN + KK * OPD  # combined input row: x window + weights


def build_program():
    nc = _PinnedActBacc("TRN2", target_bir_lowering=False)
    xin_d = nc.dram_tensor("xin", [CIN, XIN_LEN], F32, kind="ExternalInput")
    out_d = nc.dram_tensor("out", [CORE_PIX, NCH], F32, kind="ExternalOutput")

    with TileContext(nc) as tc:
        with (
            tc.tile_pool(name="const", bufs=1) as const,
            tc.tile_pool(name="pbig", bufs=1) as pbig,
            tc.tile_pool(name="pbig32", bufs=1) as pbig32,
            tc.tile_pool(name="tbig", bufs=1) as tbig,
            tc.tile_pool(name="small", bufs=2) as small,
            tc.tile_pool(name="outp", bufs=2) as outp,
            tc.tile_pool(name="psum_p", bufs=1, space="PSUM") as psum_p,
            tc.tile_pool(name="psum_s", bufs=1, space="PSUM") as psum_s,
        ):
            xin = const.tile([CIN, XIN_LEN], F32)
            # split the load across DMA queues; weights chunk first
            nc.sync.dma_start(out=xin[:, XW_LEN:], in_=xin_d[:, XW_LEN:])
            nchunk = 3
            cs = (XW_LEN + nchunk - 1) // nchunk
            for ci in range(nchunk):
                lo, hi = ci * cs, min((ci + 1) * cs, XW_LEN)
                nc.sync.dma_start(out=xin[:, lo:hi], in_=xin_d[:, lo:hi])
            xw = xin[:, :XW_LEN]
            wm = xin[:, XW_LEN:]
            eps_t = const.tile([TILE, 1], F32, tag="eps")
            nc.vector.memset(eps_t, 1e-30)
            bias_t = {}
            for val in (1.0, 81.0):
                bt = const.tile([TILE, 1], F32, tag=f"bias{int(val)}")
                nc.vector.memset(bt, val)
                bias_t[val] = bt

            NG = NB * 16  # squash groups per partition across blocks

            def squash_sq(v, sfx, split=False):
                """|s|^2 per (block, group): Square (ACT) + reduce_d (DVE).
                v: [TILE, NB*OPD]-shaped AP (any space). split=True runs it
                per block so the first block's result lands earlier."""
                v2 = small.tile([TILE, NB * OPD], F32, tag="v2" + sfx)
                sq = small.tile([TILE, NG], F32, tag="sq" + sfx)
                sqv = sq.rearrange("p (b g) -> p b g", b=NB)
                v2v = v2.rearrange("p (b g d) -> p b g d", b=NB, d=LOUT)
                vv = v.rearrange("p (b gd) -> p b gd", b=NB)
                for b in range(NB) if split else (slice(None),):
                    nc.scalar.activation(
                        out=v2v[:, b] if split else v2, in_=vv[:, b] if split else v,
                        func=AF.Square,
                    )
                    yield
                    nc.vector.tensor_reduce(
                        out=sqv[:, b] if split else sq,
                        in_=v2v[:, b] if split
                        else v2.rearrange("p (g d) -> p g d", d=LOUT),
                        axis=AX.X, op=ALU.add,
                    )
                    yield
                return sq

            def squash_tail(v, sq, denom_bias, sfx, o_engine=None):
                """Given v = c*s (c = sqrt(denom_bias)) and sq = |v|^2,
                returns outputs = squash(s) = v * sqrt(u)/(u + denom_bias).
                All ACT funcs (Square/Ln/Exp) share one HW table."""
                # g = sqrt(u)/(u+denom_bias) = exp(0.5*ln(u+eps) - ln(u+denom))
                la = small.tile([TILE, NG], F32, tag="la" + sfx)
                nc.scalar.activation(out=la, in_=sq, func=AF.Ln, bias=eps_t[:, :])
                lb = small.tile([TILE, NG], F32, tag="lb" + sfx)
                nc.scalar.activation(
                    out=lb, in_=sq, func=AF.Ln, bias=bias_t[denom_bias][:, :]
                )
                yield
                cc = small.tile([TILE, NG], F32, tag="cc" + sfx)
                nc.vector.scalar_tensor_tensor(
                    out=cc, in0=la, scalar=0.5, in1=lb,
                    op0=ALU.mult, op1=ALU.subtract,
                )
                g = small.tile([TILE, NG], F32, tag="g" + sfx)
                nc.scalar.activation(out=g, in_=cc, func=AF.Exp)
                yield
                o = small.tile([TILE, NB * OPD], F32, tag="o" + sfx)
                (o_engine or nc.vector).tensor_mul(
                    o.rearrange("p (g d) -> p g d", d=LOUT),
                    v.rearrange("p (g d) -> p g d", d=LOUT),
                    g.unsqueeze(2).to_broadcast([TILE, NG, LOUT]),
                )
                yield
                return o

            def squash(v, denom_bias, sfx, o_engine=None):
                sq = yield from squash_sq(v, sfx)
                o = yield from squash_tail(v, sq, denom_bias, sfx, o_engine)
                return o

            def logits_contrib(psb, o, sfx):
                """sum_d priors * outputs -> [TILE, NB*144] laid out (b,k,op).
                Multiplies on GPSIMD (one per block) pipelined against DVE
                reduces; f32 throughout (logits are precision-sensitive)."""
                t = tbig.tile([TILE, NB, KK, OPD], F32, tag="tg" + sfx)
                lr = small.tile([TILE, NB * KK * 16], F32, tag="lr" + sfx)
                lrv = lr.rearrange("p (b k g) -> p b k g", b=NB, k=KK)
                ov = o.rearrange("p (b gd) -> p b gd", b=NB)
                KH = 5
                for b in range(NB):
                    for k0, k1 in ((0, KH), (KH, KK)):
                        nc.gpsimd.tensor_mul(
                            t[:, b, k0:k1],
                            psb[:, b, k0:k1],
                            ov[:, b].unsqueeze(1)
                            .to_broadcast([TILE, k1 - k0, OPD]),
                        )
                        yield
                        nc.vector.tensor_reduce(
                            out=lrv[:, b, k0:k1],
                            in_=t[:, b, k0:k1].rearrange(
                                "p k (g d) -> p k g d", d=LOUT
                            ),
                            axis=AX.X, op=ALU.add,
                        )
                        yield
                return lr

            def softmax_k(lg, sfx):
                """softmax over k of [TILE, NB*144] in (b, k, op) layout."""
                e = small.tile([TILE, NB * KK * 16], F32, tag="e" + sfx)
                nc.scalar.activation(out=e, in_=lg, func=AF.Exp)
                yield
                z = small.tile([TILE, NG], F32, tag="z" + sfx)
                nc.vector.tensor_reduce(
                    out=z,
                    in_=e.rearrange("p (b k g) -> p b g k", b=NB, k=KK),
                    axis=AX.X, op=ALU.add,
                )
                zr = small.tile([TILE, NG], F32, tag="zr" + sfx)
                nc.vector.reciprocal(out=zr, in_=z)
                yield
                # probs stored fp16 with k innermost: [b, g, k] so the
                # weighted multiply runs in the DVE 2x packed mode
                pr = small.tile([TILE, NB, 16, KK], F16, tag="pr" + sfx)
                nc.vector.tensor_mul(
                    pr.rearrange("p b g k -> p b k g"),
                    e.rearrange("p (b k g) -> p b k g", b=NB, k=KK),
                    zr.rearrange("p (b g) -> p b g", b=NB)
                    .unsqueeze(2)
                    .to_broadcast([TILE, NB, KK, 16]),
                )
                yield
                return pr

            def weighted_s(psb, pr, sfx):
                """sum_k probs * priors -> [TILE, NB*256]. Both multiply
                operands are fp16 with unit-stride innermost k, so the DVE
                runs its 2x packed mode; k-sum via fp16 pairwise tree over
                the contiguous innermost axis."""
                t = tbig.tile([TILE, NB, 16, LOUT, KK], F16, tag="tt" + sfx)
                tm = t.rearrange("p b g d k -> p (b g) d k")
                nc.vector.tensor_mul(
                    tm,
                    psb.rearrange("p b g d k -> p (b g) d k"),
                    pr.rearrange("p b g k -> p (b g) k")
                    .unsqueeze(2)
                    .to_broadcast([TILE, NB * 16, LOUT, KK]),
                )
                yield
                u1 = tbig.tile([TILE, NB, 16, LOUT, 4], F16, tag="u1" + sfx)
                u1m = u1.rearrange("p b g d k -> p (b g) d k")
                nc.vector.tensor_add(u1m, tm[:, :, :, 0:4], tm[:, :, :, 4:8])
                yield
                u2 = tbig.tile([TILE, NB, 16, LOUT, 2], F16, tag="u2" + sfx)
                u2m = u2.rearrange("p b g d k -> p (b g) d k")
                nc.vector.tensor_add(u2m, u1m[:, :, :, 0:2], u1m[:, :, :, 2:4])
                yield
                u3 = tbig.tile([TILE, NB, 16, LOUT], F16, tag="u3" + sfx)
                u3m = u3.rearrange("p b g d -> p (b g) d")
                nc.vector.tensor_add(u3m, u2m[:, :, :, 0], u2m[:, :, :, 1])
                yield
                v = small.tile([TILE, NB * OPD], F32, tag="v" + sfx)
                nc.vector.tensor_add(
                    v.rearrange("p (bg d) -> p bg d", d=LOUT),
                    u3m,
                    tm[:, :, :, 8],
                )
                yield
                return v

            def tile_body(st, sfx):
                # ---- tap-sums s0 for both blocks (iter-0 needs only these) --
                s0 = psum_s.tile([TILE, NB, OPD], F32, tag="s0" + sfx)
                for b in range(NB):
                    t = st * NB + b
                    for k in range(KK):
                        dj, dk = divmod(k, 3)
                        off = 59 + t * TILE + (dj - 1) * HP + (dk - 1)
                        nc.tensor.matmul(
                            s0[:, b],
                            xw[:, off:off + TILE],
                            wm[:, k * OPD:(k + 1) * OPD],
                            start=(k == 0), stop=(k == KK - 1),
                        )
                        yield
                sq0 = yield from squash_sq(
                    s0.rearrange("p b gd -> p (b gd)"), sfx, split=True
                )
                # ---- per-tap priors, block by block through the shared PSUM
                # slot; ACT copies each block out to SBUF fp32 ----
                # two priors copies: f32 [b,k,g,d] for the precision-
                # sensitive logits path, fp16 k-innermost [b,g,d,k] for the
                # 2x-mode weighted multiplies / contiguous k-trees
                psb32 = pbig32.tile([TILE, NB, KK, OPD], F32, tag="q" + sfx)
                psb16 = pbig.tile(
                    [TILE, NB, 16, LOUT, KK], F16, tag="psb" + sfx
                )
                for b in range(NB):
                    t = st * NB + b
                    pp = psum_p.tile([TILE, KK, OPD], F32, tag="pp")
                    for k in range(KK):
                        dj, dk = divmod(k, 3)
                        off = 59 + t * TILE + (dj - 1) * HP + (dk - 1)
                        nc.tensor.matmul(
                            pp[:, k, :],
                            xw[:, off:off + TILE],
                            wm[:, k * OPD:(k + 1) * OPD],
                            start=True, stop=True,
                        )
                        yield
                    nc.scalar.copy(out=psb32[:, b], in_=pp)
                    yield
                    nc.scalar.copy(
                        out=psb16[:, b].rearrange("p g d k -> p k g d"),
                        in_=psb32[:, b].rearrange(
                            "p k (g d) -> p k g d", d=LOUT
                        ),
                    )
                    yield

                # ---- routing iter 0: probs uniform, s = s0/9; squash folds
                # the 1/9 via denom_bias=81 ----
                o0 = yield from squash_tail(
                    s0.rearrange("p b gd -> p (b gd)"), sq0, 81.0, sfx
                )
                l1 = yield from logits_contrib(psb32, o0, sfx)
                # ---- iter 1 ----
                pr1 = yield from softmax_k(l1, sfx)
                v1 = yield from weighted_s(psb16, pr1, sfx)
                o1 = yield from squash(v1, 1.0, sfx, o_engine=nc.gpsimd)
                l2c = yield from logits_contrib(psb32, o1, sfx)
                l2 = small.tile([TILE, NB * KK * 16], F32, tag="l2" + sfx)
                nc.vector.tensor_add(l2, l1, l2c)
                yield
                # ---- iter 2 ----
                pr2 = yield from softmax_k(l2, sfx)
                v2 = yield from weighted_s(psb16, pr2, sfx)
                o2 = yield from squash(v2, 1.0, sfx)
                # ---- sum over input planes p, store [pix, ch] rows ----
                r = outp.tile([TILE, NB, NCH], F32, tag="rr" + sfx)
                nc.vector.tensor_reduce(
                    out=r,
                    in_=o2.rearrange(
                        "p (b o q d) -> p b o d q", b=NB, o=POUT, q=PIN
                    ),
                    axis=AX.X, op=ALU.add,
                )
                yield
                nc.sync.dma_start(
                    out=out_d[st * NB * TILE:(st + 1) * NB * TILE, :]
                    .rearrange("(b p) c -> p b c", b=NB),
                    in_=r,
                )

            # Interleave instruction emission with a sliding window of two
            # super-tiles so each engine's in-order queue alternates between
            # independent dependency chains.
            gens = []
            nxt = 0
            while gens or nxt < NST:
                while len(gens) < 2 and nxt < NST:
                    gens.append(tile_body(nxt, "AB"[nxt % 2]))
                    nxt += 1
                for gn in list(gens):
                    try:
                        next(gn)
                    except StopIteration:
                        gens.remove(gn)
    nc.compile()
    return nc


_PROG = None


def _get_prog():
    global _PROG
    if _PROG is None:
        _PROG = build_program()
    return _PROG


def _make_inputs(x, weight):
    # block-diagonal moving weights: [c=(p,l), (k, o, p, d)]
    wmov = np.zeros((CIN, KK, POUT, PIN, LOUT), np.float32)
    for p in range(PIN):
        # rows p*LIN..p*LIN+LIN-1 hold weight[o, p, k, l, d]
        wmov[p * LIN:(p + 1) * LIN, :, :, p, :] = np.transpose(
            weight[:, p], (2, 1, 0, 3)
        )  # (l, k, o, d) from (o, k, l, d)
    wmov = wmov.reshape(CIN, KK * OPD)

    xp = np.pad(x, ((0, 0), (0, 0), (1, 1), (1, 1))).reshape(4, CIN, NPIX)
    xpm = np.pad(xp, ((0, 0), (0, 0), (64, 64)))
    in_maps = []
    for c in range(8):
        n, half = divmod(c, 2)
        p0 = 0 if half == 0 else P0_B
        lo = 64 + p0 - 59
        xin = np.concatenate([xpm[n][:, lo:lo + XW_LEN], wmov], axis=1)
        in_maps.append({"xin": np.ascontiguousarray(xin)})
    return in_maps


def _assemble(results):
    out = np.empty((4, NCH, 56, 56), np.float32)
    for n in range(4):
        full = np.empty((NCH, NPIX), np.float32)
        full[:, :CORE_PIX] = results[2 * n]["out"].T
        full[:, CORE_PIX:] = results[2 * n + 1]["out"].T[:, CORE_PIX - P0_B:]
        out[n] = full.reshape(NCH, HP, HP)[:, 1:57, 1:57]
    return out


def kernel(x, weight):
    x = np.asarray(x, np.float32)
    weight = np.asarray(weight, np.float32)
    in_maps = _make_inputs(x, weight)
    last_err = None
    for _ in range(3):  # retry transient NRT/device errors
        try:
            res = run_bass_kernel_spmd(
                _get_prog(), in_maps, core_ids=list(range(8))
            )
            return _assemble(res.results)
        except Exception as e:  # noqa: BLE001
            last_err = e
    raise last_err


if __name__ == "__main__":
    rng = np.random.default_rng(0)
    x = rng.standard_normal((4, 32, 56, 56), dtype=np.float32)
    w = rng.standard_normal((4, 4, 9, 8, 16), dtype=np.float32)
    y = kernel(x, w)
    print("out", y.shape, y.dtype, float(np.abs(y).mean()))
